# revision 28
# baseline (speedup 1.0000x reference)
"""Multi-hash embedding lookup on Trainium2 (Bass/Tile), 8 NeuronCores.

Reference computation (per token id):
    idx[h] = hash_tables[id, h] * (id != 0)        h in {0, 1}
    out    = p[id, 0] * W[idx[0]] + p[id, 1] * W[idx[1]]

Sharding: data parallel over batch. Each of the 8 cores handles 128 of
the 1024 batch rows; W / hash_tables / p are replicated per core.

Host prep (layout/dtype transforms only, no data-dependent indexing):
  * meta[v] = (f32(hash_tables[v,0]), f32(hash_tables[v,1]), p[v,0], p[v,1])
    with meta[0, 0:2] = 0 so the (id == 0) -> bucket 0 masking needs no
    on-device compute (id 0 is the only id that reads row 0). Hash indices
    are < 2^24, hence exact in f32. Rows are packed 32-per-512B-super-row
    (meta32[r] covers vocab ids [32r, 32r+32)) so that the super-row index
    id >> 5 < 2^15 fits the int16 indices of the gpsimd dma_gather
    instruction.
  * W is converted to bf16 (halves the gathered-row HBM traffic and lets
    the scale/pair-sum run at 2x DVE throughput; norm rel-err ~0.5%, well
    under the 2e-2 gate). The weighted sum is computed and stored as bf16;
    the host upcasts the returned output to f32.
  * ids for a core are laid out [128, 200]: partition p holds batch row
    c*128+p, so each partition's S=200 output rows are contiguous in DRAM
    (2.5KB+ per-partition store runs). A copy of ids in dma_gather's
    wrapped index layout ([16, n/16] per group, replicated across
    partition 16-groups) is shipped as `idsw`.

Device kernel per core (tokens T = 25600, processed in NIT groups of
G columns; the next groups' meta dma_gathers are emitted ahead of this
group's W gathers so their DVE select chains overlap the emission):
  1. one dma_gather per group fetches each token's meta super-row
     (512B, one SWDGE instruction per group, indices = ids >> 5 as int16,
     computed on device). Striped over SWDGE queues 1..3 so they never
     serialize behind the W-row indirect DMAs pinned to queue 0.
  2. the 16B meta entry is selected out of each 512B super-row with an
     exact f32 one-hot (is_equal on id & 31 vs an iota) multiply +
     add-reduce on the vector engine; hash indices are converted f32 ->
     int32 exactly; pv is converted to bf16.
  3. the W rows are fetched with SWDGE vector-indirect DMAs. The HW
     honors exactly one index per destination partition row (verified:
     multi-index-per-partition scrambles descriptors), so each
     instruction moves 128 rows x 256B; 2 hashes x G columns such
     instructions per group.
  4. emb *= pv (broadcast along E, one 4D DVE op, bf16), pair-sum over
     the two hashes (one strided DVE add, bf16), and one contiguous
     store of the group's output rows (G*256B per partition run).
"""

import numpy as np
import ml_dtypes

from concourse import bass, mybir
import concourse.bacc as bacc
import concourse.tile as tile
import concourse.bass_utils as bass_utils

VOCAB = 1_000_001
NBP1 = 500_001           # num_buckets + 1 (row 0 = zero row)
E = 128                  # embedding dim
B, S = 1024, 200
NCORES = 8
BL = B // NCORES         # 128 batch rows per core
T = BL * S               # 25600 tokens per core
P = 128                  # SBUF partitions
NCOL = S                 # 200 id columns per core (partition = batch row)
G = 10                   # columns per group
PACK = 32                # vocab entries per meta super-row (dma_gather int16 range)
NSUP = (VOCAB + PACK - 1) // PACK  # 31251 super-rows (< 2^15)

_cached_nc = None
last_results = None      # BassKernelResults from the most recent run


def _build_program(G=G, WBUFS=8, PFD=2, NQ=1):
    nc = bacc.Bacc(
        "TRN2", target_bir_lowering=False, debug=False, enable_asserts=False,
        num_swdge_queues=NQ,
    )
    # host ships the wrapped, pre-shifted super-row indices (int16) and the
    # in-super-row slots ids & 31 (int32) -- both pure layout transforms
    ids16_d = nc.dram_tensor(
        "ids16", [P, NCOL * 8], mybir.dt.int16, kind="ExternalInput"
    ).ap()
    amod_d = nc.dram_tensor(
        "amod", [P, NCOL], mybir.dt.int32, kind="ExternalInput"
    ).ap()
    # packed meta words (idx | pv13 << 19). Gathered as int32 (fewest
    # elements -- Pool cost is element-count-priced) but selected as int16
    # halves: every int16 round-trips the DVE's fp32 ALU exactly, so the
    # one-hot mult+add select is bit-exact (full int32 words would lose low
    # bits past the 24-bit f32 mantissa)
    meta_d = nc.dram_tensor(
        "meta32", [NSUP, 2 * PACK], mybir.dt.int32, kind="ExternalInput"
    ).ap()
    w_d = nc.dram_tensor("w", [NBP1, E], mybir.dt.bfloat16, kind="ExternalInput").ap()
    out_d = nc.dram_tensor("out", [T, E], mybir.dt.bfloat16, kind="ExternalOutput").ap()
    # flat per-partition view: partition p covers out rows [p*S, (p+1)*S)
    out_flat = out_d.rearrange("(p s) e -> p (s e)", p=P)

    groups = _group_list(G)
    starts = np.cumsum([0] + groups).tolist()
    nit = len(groups)
    with tile.TileContext(nc) as tc:
        with tc.tile_pool(name="idsp", bufs=1) as idsp, tc.tile_pool(
            name="work", bufs=WBUFS
        ) as wp:
            # wrapped-layout super-row indices (one [16, g*8] block per group,
            # replicated over all partition-16-groups), pre-shifted on host.
            # Loaded in two chunks so the first groups' gathers start early.
            ids16 = idsp.tile([P, NCOL * 8], mybir.dt.int16)
            split = starts[min(PFD, nit)] * 8
            nc.sync.dma_start(out=ids16[:, :split], in_=ids16_d[:, :split])
            nc.sync.dma_start(out=ids16[:, split:], in_=ids16_d[:, split:])
            amod_sb = idsp.tile([P, NCOL], mybir.dt.int32)
            nc.sync.dma_start(out=amod_sb[:], in_=amod_d[:])
            iota_t = idsp.tile([P, PACK], mybir.dt.int32)
            nc.gpsimd.iota(iota_t[:], pattern=[[1, PACK]], base=0, channel_multiplier=0)
            supers = {}

            def emit_meta_gather(it):
                # (a) one dma_gather fetches the 256B meta super-row of every
                # token in the group: super[p, g, :] = meta32[ids[p, c0+g] >> 5]
                g = groups[it]
                super_t = wp.tile([P, G * 2 * PACK], mybir.dt.int32, tag="super")
                nc.gpsimd.dma_gather(
                    out_ap=super_t[:, : g * 2 * PACK].rearrange(
                        "p (g e) -> p g e", e=2 * PACK
                    ),
                    in_ap=meta_d[:],
                    idxs_ap=ids16[:, starts[it] * 8 : starts[it + 1] * 8],
                    num_idxs=g * P,
                    num_idxs_reg=g * P,
                    elem_size=2 * PACK,
                    # >64 descriptors per ring: must not pack as one packet
                    single_packet=False,
                    # stripe meta fetches over the other SWDGE queues (their
                    # own Q7 cpu pairs) so they never serialize behind the
                    # W-row indirect DMAs pinned to queue 0
                    queue_num=(1 + it % (NQ - 1)) if NQ > 1 else 0,
                )
                supers[it] = super_t

            for k in range(min(PFD, nit)):
                emit_meta_gather(k)
            for it in range(nit):
                g = groups[it]
                c0 = starts[it]
                if it + PFD < nit:
                    # upcoming groups' meta fetches ahead of this group's
                    # W gathers so their DVE selects overlap the emission
                    emit_meta_gather(it + PFD)
                super_t = supers.pop(it)
                # (b,c) one-hot position of each token inside its super-row:
                # cmpi[p,g,k] = int16((ids[p,g] & 31) == k). Each packed entry
                # is two words idx | pv13 << 19 held as four int16 halves; the
                # 0/1-mask mult + add-reduce select is exact on int16 lanes.
                cmpi_t = wp.tile([P, G * PACK], mybir.dt.int16, tag="cmpi")
                cmpi3 = cmpi_t[:, : g * PACK].rearrange("p (g k) -> p g k", k=PACK)
                nc.vector.tensor_tensor(
                    out=cmpi3,
                    in0=amod_sb[:, c0 : c0 + g].unsqueeze(2).to_broadcast([P, g, PACK]),
                    in1=iota_t[:].unsqueeze(1).to_broadcast([P, g, PACK]),
                    op=mybir.AluOpType.is_equal,
                )
                # (d) mask the super-rows in place, (e) add-reduce the 32
                # candidate slots -> sel16[p, g, 0:4] = halves (lo0,hi0,lo1,hi1)
                super16 = super_t[:, : g * 2 * PACK].bitcast(mybir.dt.int16)
                super2 = super16.rearrange("p (g k h) -> p g k h", k=PACK, h=4)
                nc.vector.tensor_tensor(
                    out=super2,
                    in0=super2,
                    in1=cmpi3.unsqueeze(3).to_broadcast([P, g, PACK, 4]),
                    op=mybir.AluOpType.mult,
                )
                sel_t = wp.tile([P, 4 * G], mybir.dt.int16, tag="sel")
                with nc.allow_low_precision(
                    reason="add of one-hot-masked int16 lanes is exact"
                ):
                    nc.vector.tensor_reduce(
                        out=sel_t[:, : 4 * g].rearrange("p (g h) -> p g h", h=4),
                        in_=super16.rearrange(
                            "p (g k h) -> p g h k", k=PACK, h=4
                        ),
                        axis=mybir.AxisListType.X,
                        op=mybir.AluOpType.add,
                    )
                # little-endian halves reassemble to the packed int32 words
                selw = sel_t[:, : 4 * g].bitcast(mybir.dt.int32)
                # unpack: idx = w & (2^19 - 1); pv13 = w >>> 19 is a windowed
                # mini-float (1s 5e 7m, e=0 <-> 2^-15) re-expanded to f32 bits
                # as s<<31 | (e5+112)<<23 | m<<16 -- the exponent rebias is
                # added BEFORE the shift so every arithmetic stays < 2^24
                idxi_t = wp.tile([P, 2 * G], mybir.dt.int32, tag="idxi")
                nc.vector.tensor_scalar(
                    out=idxi_t[:, : 2 * g],
                    in0=selw,
                    scalar1=(1 << 19) - 1,
                    scalar2=None,
                    op0=mybir.AluOpType.bitwise_and,
                )
                pvs_t = wp.tile([P, 2 * G], mybir.dt.int32, tag="pvs")
                nc.vector.tensor_scalar(
                    out=pvs_t[:, : 2 * g],
                    in0=selw,
                    scalar1=19,
                    scalar2=0x0FFF,
                    op0=mybir.AluOpType.logical_shift_right,
                    op1=mybir.AluOpType.bitwise_and,
                )
                # (x + rebias) * 2^16: exact in the f32 ALU (15-bit value with
                # 16 trailing zero bits), equivalent to the left shift
                nc.vector.tensor_scalar(
                    out=pvs_t[:, : 2 * g],
                    in0=pvs_t[:, : 2 * g],
                    scalar1=112 << 7,
                    scalar2=1 << 16,
                    op0=mybir.AluOpType.add,
                    op1=mybir.AluOpType.mult,
                )
                sgn_t = wp.tile([P, 2 * G], mybir.dt.int32, tag="sgn")
                nc.vector.tensor_scalar(
                    out=sgn_t[:, : 2 * g],
                    in0=selw,
                    scalar1=-(1 << 31),
                    scalar2=None,
                    op0=mybir.AluOpType.bitwise_and,
                )
                nc.vector.tensor_tensor(
                    out=pvs_t[:, : 2 * g],
                    in0=pvs_t[:, : 2 * g],
                    in1=sgn_t[:, : 2 * g],
                    op=mybir.AluOpType.bitwise_or,
                )
                # pv as bf16 for the 2x-throughput scale/sum below
                pvb_t = wp.tile([P, 2 * G], mybir.dt.bfloat16, tag="pvb")
                nc.vector.tensor_copy(
                    out=pvb_t[:, : 2 * g],
                    in_=pvs_t[:, : 2 * g].bitcast(mybir.dt.float32),
                )
                emb_t = wp.tile([P, 2 * G * E], mybir.dt.bfloat16, tag="emb")
                for j in range(g):
                    for h in range(2):
                        nc.gpsimd.indirect_dma_start(
                            out=emb_t[:, (2 * j + h) * E : (2 * j + h + 1) * E],
                            out_offset=None,
                            in_=w_d[:],
                            in_offset=bass.IndirectOffsetOnAxis(
                                ap=idxi_t[:, 2 * j + h : 2 * j + h + 1], axis=0
                            ),
                        )
                # emb[p, g, h, :] *= pv[p, g, h]; o = emb[:, :, 0, :] + emb[:, :, 1, :]
                emb4 = emb_t[:, : 2 * g * E].rearrange(
                    "p (g h e) -> p g h e", h=2, e=E
                )
                pv_bc = (
                    pvb_t[:, : 2 * g]
                    .rearrange("p (g h) -> p g h", h=2)
                    .unsqueeze(3)
                    .to_broadcast([P, g, 2, E])
                )
                nc.vector.tensor_tensor(
                    out=emb4, in0=emb4, in1=pv_bc, op=mybir.AluOpType.mult
                )
                o_t = wp.tile([P, G * E], mybir.dt.bfloat16, tag="o")
                nc.vector.tensor_add(
                    out=o_t[:, : g * E],
                    in0=emb4[:, :, 0, :],
                    in1=emb4[:, :, 1, :],
                )
                # store: partition p's g output rows are contiguous in DRAM
                # (out row p*S + c0 + j) -> one g*256B run per partition
                nc.sync.dma_start(
                    out=out_flat[:, c0 * E : (c0 + g) * E],
                    in_=o_t[:, : g * E],
                )
    nc.compile()
    return nc


def _group_list(Gmax):
    """Column-group sizes: Gmax-wide groups with short tapers at both ends.
    The front taper makes the first group's DVE select (which gates the
    first W gathers) cheap; the tail taper makes the post-last-Pool-
    instruction drain (last group's DVE + store chain) cheap."""
    front = [2, 8]
    tail = [8, 2]
    body = NCOL - sum(front) - sum(tail)
    assert body % Gmax == 0
    return front + [Gmax] * (body // Gmax) + tail


def _pack_pv13(p):
    """f32 -> 13-bit minifloat (1s 5e 7m), exponent window [2^-15, 2^16],
    round-to-nearest (carry propagates into the exponent), underflow clamps
    to the window floor (~3e-5 -- negligible for N(0,1) weights)."""
    b = p.view(np.uint32).astype(np.uint64)
    br = b + 0x8000  # round mantissa bit 16
    exp8 = (br >> 23) & 0xFF
    e5 = np.clip(exp8.astype(np.int64) - 112, 0, 31).astype(np.uint64)
    m7 = np.where(exp8 < 112, 0, (br >> 16) & 0x7F)
    s = (b >> 31) & 1
    return (s << 12 | e5 << 7 | m7).astype(np.uint32)


def _host_prep_shared(W, p, hash_tables):
    # meta word per (vocab word, hash): idx | pv13 << 19. idx < 2^19 covers
    # the 500001 buckets; pv13 is a 13-bit minifloat with bf16's 7-bit
    # mantissa. 32 two-word entries pack per 256B super-row so the super-row
    # index (id >> 5, < 2^15) fits dma_gather's int16 indices.
    words = np.zeros((NSUP * PACK, 2), dtype=np.uint32)
    words[:VOCAB] = hash_tables.astype(np.uint32)
    words[0] = 0  # id 0 -> bucket 0 (the masking in the reference)
    words[:VOCAB] |= _pack_pv13(np.ascontiguousarray(p)) << 19
    meta32 = words.view(np.int32).reshape(NSUP, 2 * PACK)
    wbf = W.astype(ml_dtypes.bfloat16)
    return {"meta32": meta32, "w": wbf}


def _host_prep_core(ids, shared, core):
    # partition p holds batch row core*BL + p; columns are seq positions
    ids_dev = np.ascontiguousarray(ids[core * BL : (core + 1) * BL])
    # wrapped idx layout for dma_gather: ordinal o = g*128 + p of group
    # `it` sits at [o % 16, it*G*8 + o // 16]; replicated across all
    # 16-partition groups (each SWDGE queue cpu pair reads its own copy).
    # Shipped pre-shifted (>> 5) as int16.
    blocks = []
    c0 = 0
    for g in _group_list(G):
        vals = ids_dev[:, c0 : c0 + g].T.reshape(-1) >> 5
        blocks.append(vals.reshape(g * 8, 16).T)
        c0 += g
    ids16 = np.tile(np.concatenate(blocks, axis=1), (8, 1)).astype(np.int16)
    ids16 = np.ascontiguousarray(ids16)
    amod = np.ascontiguousarray(ids_dev & (PACK - 1))
    return {"ids16": ids16, "amod": amod, **shared}


def _host_prep(ids, W, p, hash_tables, core):
    return _host_prep_core(ids, _host_prep_shared(W, p, hash_tables), core)


def kernel(ids, W, p, hash_tables):
    global _cached_nc, last_results
    if _cached_nc is None:
        _cached_nc = _build_program()
    nc = _cached_nc

    ids = np.ascontiguousarray(np.asarray(ids), dtype=np.int32)
    W = np.ascontiguousarray(np.asarray(W), dtype=np.float32)
    p = np.ascontiguousarray(np.asarray(p), dtype=np.float32)
    hash_tables = np.ascontiguousarray(np.asarray(hash_tables), dtype=np.int32)

    shared = _host_prep_shared(W, p, hash_tables)
    in_maps = [_host_prep_core(ids, shared, c) for c in range(NCORES)]

    last_results = bass_utils.run_bass_kernel_spmd(
        nc, in_maps, core_ids=list(range(NCORES))
    )

    out = np.empty((B, S, E), dtype=np.float32)
    for c in range(NCORES):
        ob = last_results.results[c]["out"].view(ml_dtypes.bfloat16)
        out[c * BL : (c + 1) * BL] = ob.astype(np.float32).reshape(BL, S, E)
    return out
